# revision 83
# baseline (speedup 1.0000x reference)
"""Trainium2 Bass kernel for nn_CrossAttentionDown (region-RoPE cross attention).

Full-input contract: kernel(**inputs) takes the complete tensors, shards
(B, H) across 8 NeuronCores (each core: one batch, half the heads), runs an
SPMD Bass kernel, and gathers the full [B, H, P, D] output.

Math notes (vs the jax reference):
 - softmax(x + c) == softmax(x) per row, so the per-head bias_diff constant
   drops out; only delta_h = bias_same - bias_diff matters. It rides the QK^T
   contraction: K side gets onehot(regions[t]==n), Q side delta_h*onehot(p//4==n).
 - The 128-dim QK contraction per head is
     [ k_h1*cos (32) | k_h1*sin (32) | rot(k_h2) (32) | region-onehot (32) ]
   paired with Q rows
     [ q'_h1 (32) | swapneg(q'_h1) (32) | q'_h2 (32) | delta_h*onehotP (32) ]
   where q' is the rotated (and 1/sqrt(D)-scaled) query. The first-half RoPE
   on K is "doubled" into plain cos/sin products (signs live on the Q side);
   the second half is rotated classically using a reversed-stride pair-swap
   view (packed, so the DVE 2x mode applies).
 - t is processed in a stride-32 permutation: tile l covers t in {32p+l}.
   This makes both K and V HBM loads fully contiguous (8KB per partition
   line), and V's tile l is just a column slice of the contiguous load.
 - Scores are computed [t, p] per tile; exp(scores) feeds AV as the matmul
   stationary with V moving, so the output lands directly as [p, d]. The
   softmax denominator comes from a ones-column appended to V.
"""

import sys

if "/opt/trn_rl_repo" not in sys.path:
    sys.path.insert(0, "/opt/trn_rl_repo")

import math

import numpy as np

B, H, T, D = 4, 16, 4096, 64
MAX_N = 32
R_TOK = 4
P = MAX_N * R_TOK  # 128 pool queries
NCORES = 8
HPC = H // 2  # heads per core
NT = T // 128  # 32 t-tiles
TL = 32  # t-rows per partition in contiguous layout
DH = D // 2  # 32 dims per rope half
JH = DH // 2  # 16 rotation pairs per half
THETA = 10000.0

_cache = {}


def _split_waits(nc, maxw=1):
    """The pinned walrus rejects instructions with more than one embedded
    semaphore wait. Hoist excess waits into preceding wait-only Drain
    instructions on the same engine (same-engine program order preserves
    the blocking semantics)."""
    import concourse.mybir as mybir

    n_new = 0
    for f in nc.m.functions:
        for blk in f.blocks:
            new_list = []
            for inst in blk.instructions:
                si = getattr(inst, "sync_info", None)
                waits = list(si.on_wait) if si is not None and si.on_wait else []
                if len(waits) > maxw:
                    excess, keep = waits[:-maxw], waits[-maxw:]
                    for j, w in enumerate(excess):
                        d = mybir.InstDrain(name=f"{inst.name}-w{j}", ins=[], outs=[])
                        d.engine = inst.engine
                        d.sync_info = mybir.SyncInfo(on_wait=[w], on_update=[])
                        d.debug = inst.debug
                        new_list.append(d)
                        n_new += 1
                    si.on_wait = keep
                new_list.append(inst)
            blk.instructions[:] = new_list
    return n_new


def _emit_range_reduce(nc, mybir, pool, ang, ncols, name, eng=None):
    """In-place reduce ang (>=0) to [-pi, pi] mod 2pi. Two-term Cody-Waite:
    hi=6.28125 (k*hi exact in fp32 for k<=2^17/201), lo=2pi-hi. Robust to
    either float->int convert rounding mode via the conditional steps."""
    f32 = mybir.dt.float32
    i32 = mybir.dt.int32
    INV2PI = float(np.float32(1.0 / (2.0 * math.pi)))
    HI = 6.28125
    LO = float(np.float32(2.0 * math.pi - HI))
    PI = float(np.float32(math.pi))
    if eng is None:
        eng = nc.vector
    kf = pool.tile([128, ncols], f32, name=f"{name}_kf", tag=f"{name}_kf")
    ki = pool.tile([128, ncols], i32, name=f"{name}_ki", tag=f"{name}_ki")
    mt = pool.tile([128, ncols], f32, name=f"{name}_mt", tag=f"{name}_mt")
    eng.tensor_scalar_mul(kf[:], ang, INV2PI)
    eng.tensor_copy(ki[:], kf[:])
    eng.tensor_copy(kf[:], ki[:])
    eng.scalar_tensor_tensor(
        ang, kf[:], -HI, ang, op0=mybir.AluOpType.mult, op1=mybir.AluOpType.add
    )
    eng.scalar_tensor_tensor(
        ang, kf[:], -LO, ang, op0=mybir.AluOpType.mult, op1=mybir.AluOpType.add
    )
    TWOPI = float(np.float32(2.0 * math.pi))
    eng.tensor_scalar(mt[:], ang, PI, None, op0=mybir.AluOpType.is_gt)
    eng.scalar_tensor_tensor(
        ang, mt[:], -TWOPI, ang, op0=mybir.AluOpType.mult, op1=mybir.AluOpType.add
    )
    eng.tensor_scalar(mt[:], ang, -PI, None, op0=mybir.AluOpType.is_lt)
    eng.scalar_tensor_tensor(
        ang, mt[:], TWOPI, ang, op0=mybir.AluOpType.mult, op1=mybir.AluOpType.add
    )


def _emit_sincos(nc, mybir, pool, AF, ang, sin_out, cos_out, ncols, name, eng=None):
    """sin/cos of ang (any positive range): range-reduce then Sin, and
    cos(x) = sin(x + pi/2) with a re-reduction of the shifted angle."""
    f32 = mybir.dt.float32
    if eng is None:
        eng = nc.vector
    _emit_range_reduce(nc, mybir, pool, ang, ncols, name, eng=eng)
    nc.scalar.activation(sin_out, ang, AF.Sin)
    eng.tensor_scalar_add(ang, ang, float(math.pi / 2))
    mtc = pool.tile([128, ncols], f32, name=f"{name}_mtc", tag=f"{name}_mtc")
    eng.tensor_scalar(
        mtc[:], ang, float(np.float32(math.pi)), None, op0=mybir.AluOpType.is_gt
    )
    eng.scalar_tensor_tensor(
        ang, mtc[:], float(-2.0 * math.pi), ang,
        op0=mybir.AluOpType.mult, op1=mybir.AluOpType.add,
    )
    nc.scalar.activation(cos_out, ang, AF.Sin)


def _emit_sincos_pool(nc, mybir, pool, AF, scal, ang, sin_out, cos_out, ncols, name):
    """Pool-engine variant: only TensorTensor/TensorCopy are legal Pool
    opcodes, so every scalar op uses a broadcast constant from `scal`
    ([128, 8] = [inv2pi, -HI, -LO, pi, -pi, -2pi, 2pi, pi/2])."""
    f32 = mybir.dt.float32
    i32 = mybir.dt.int32
    eng = nc.gpsimd
    mul = mybir.AluOpType.mult
    add = mybir.AluOpType.add

    def bc(i):
        return scal[:, i : i + 1].broadcast_to([128, ncols])

    kf = pool.tile([128, ncols], f32, name=f"{name}_kf", tag=f"{name}_kf")
    ki = pool.tile([128, ncols], i32, name=f"{name}_ki", tag=f"{name}_ki")
    mt = pool.tile([128, ncols], f32, name=f"{name}_mt", tag=f"{name}_mt")
    eng.tensor_tensor(kf[:], ang, bc(0), op=mul)
    eng.tensor_copy(ki[:], kf[:])
    eng.tensor_copy(kf[:], ki[:])
    eng.tensor_tensor(mt[:], kf[:], bc(1), op=mul)  # kf * -HI
    eng.tensor_tensor(ang, ang, mt[:], op=add)
    eng.tensor_tensor(mt[:], kf[:], bc(2), op=mul)  # kf * -LO
    eng.tensor_tensor(ang, ang, mt[:], op=add)
    eng.tensor_tensor(mt[:], ang, bc(3), op=mybir.AluOpType.is_gt)
    eng.tensor_tensor(mt[:], mt[:], bc(5), op=mul)  # mask * -2pi
    eng.tensor_tensor(ang, ang, mt[:], op=add)
    eng.tensor_tensor(mt[:], ang, bc(4), op=mybir.AluOpType.is_lt)
    eng.tensor_tensor(mt[:], mt[:], bc(6), op=mul)  # mask * 2pi
    eng.tensor_tensor(ang, ang, mt[:], op=add)
    nc.scalar.activation(sin_out, ang, AF.Sin)
    eng.tensor_tensor(ang, ang, bc(7), op=add)  # + pi/2
    eng.tensor_tensor(mt[:], ang, bc(3), op=mybir.AluOpType.is_gt)
    eng.tensor_tensor(mt[:], mt[:], bc(5), op=mul)
    eng.tensor_tensor(ang, ang, mt[:], op=add)
    nc.scalar.activation(cos_out, ang, AF.Sin)


def _build_program(split_waits=True):
    import concourse.bass as bass
    import concourse.mybir as mybir
    import concourse.tile as tile

    f32 = mybir.dt.float32
    bf16 = mybir.dt.float16  # 16-bit matmul dtype (fp16: 11-bit mantissa)
    AF = mybir.ActivationFunctionType

    nc = bass.Bass("TRN2", target_bir_lowering=False, debug=False)

    q_d = nc.dram_tensor("q", [HPC, P, D], f32, kind="ExternalInput")
    k_d = nc.dram_tensor("k", [HPC, T, D], f32, kind="ExternalInput")
    v_d = nc.dram_tensor("v", [HPC, T, D], f32, kind="ExternalInput")
    reg_d = nc.dram_tensor("regions_f", [T], f32, kind="ExternalInput")
    bs_d = nc.dram_tensor("bias_same8", [HPC], f32, kind="ExternalInput")
    bd_d = nc.dram_tensor("bias_diff8", [HPC], f32, kind="ExternalInput")
    out_d = nc.dram_tensor("out", [HPC, P, D], f32, kind="ExternalOutput")

    # ---- compile-time constants (embedded in the NEFF) ----
    inv = (1.0 / (THETA ** (np.arange(0, DH, 2, dtype=np.float64) / DH))).astype(
        np.float32
    )  # [16] rope inverse frequencies (each half d=32)
    inv128_np = np.broadcast_to(inv[None, :], (128, JH)).copy()
    nids128_np = np.broadcast_to(
        np.arange(1, MAX_N + 1, dtype=np.float32)[None, :], (128, MAX_N)
    ).copy()
    ridx_np = (np.arange(128, dtype=np.float32) // R_TOK + 1.0)[:, None].copy()
    onehotP_np = (
        np.arange(MAX_N)[:, None] == (np.arange(128)[None, :] // R_TOK)
    ).astype(np.float32)
    at_prefix_np = (
        np.arange(MAX_N)[:, None] < (np.arange(128)[None, :] // R_TOK)
    ).astype(np.float32)
    # contiguous layout positions: tvals32[p, tl] = 32*p + tl
    tvals32_np = (
        32.0 * np.arange(128, dtype=np.float32)[:, None]
        + np.arange(TL, dtype=np.float32)[None, :]
    ).copy()
    # K-side h1 trig tables are pure compile-time: angle = (32p+tl)*inv_j,
    # pair-expanded to 32 dims (d=2j, 2j+1 share the pair-j coefficient).
    # Stored interleaved per tl as [cos(32) | sin(32)] so one broadcast mul
    # produces both h1 product blocks of Ka.
    tfull = tvals32_np.astype(np.float64)  # [128, TL]
    invx = np.repeat(inv.astype(np.float64), 2)  # [DH] pair-expanded
    ang1_full = tfull[:, :, None] * invx[None, None, :]  # [128, TL, DH]
    c1s1_il = np.stack(
        [np.cos(ang1_full), np.sin(ang1_full)], axis=2
    )  # [128, TL, 2, DH]
    c1s1_np = c1s1_il.reshape(128, TL * 2 * DH).astype(np.float16)
    ones_np = np.ones((128, 1), np.float32)
    ident_np = np.eye(128, dtype=np.float32)
    # sign pair [-1, +1] for the signed sin expansion (h2 swap-mul)
    sgn2_np = np.broadcast_to(
        np.array([-1.0, 1.0], np.float32)[None, :], (128, 2)
    ).copy()
    # Q-side swapneg sign pattern: [+1, -1] per pair (col 2j gets +q'[2j+1],
    # col 2j+1 gets -q'[2j] after the reversed-pair view)
    sgnq_np = np.broadcast_to(
        np.array([1.0, -1.0], np.float32)[None, :], (128, 2)
    ).copy()

    # pack the small f32 constants into single inline tensors so each lands
    # with one early DMA: cf32 [128, 62] = [inv(16) | nids(32) | ridx | ones |
    # sgn2(2) | sgnq(2) | reduce-scalars(8)]; c32 = [onehotP | atpre]
    HI_ = 6.28125
    LO_ = float(np.float32(2.0 * math.pi - HI_))
    rscal_np = np.broadcast_to(
        np.array(
            [
                1.0 / (2.0 * math.pi),
                -HI_,
                -LO_,
                math.pi,
                -math.pi,
                -2.0 * math.pi,
                2.0 * math.pi,
                math.pi / 2.0,
            ],
            np.float32,
        )[None, :],
        (128, 8),
    ).copy()
    cf32_np = np.concatenate(
        [inv128_np, nids128_np, ridx_np, ones_np, sgn2_np, sgnq_np, rscal_np],
        axis=1,
    )
    c32_np = np.concatenate([onehotP_np, at_prefix_np], axis=1)

    ident_bf_c = nc.inline_tensor(ident_np.astype(np.float16), name="ident_bf_c")
    cf32_c = nc.inline_tensor(cf32_np, name="cf32_c")
    c32_c = nc.inline_tensor(c32_np, name="c32_c")
    c1s1_c = nc.inline_tensor(c1s1_np, name="c1s1_c")

    NKV = 3  # K/V buffer depth (prefetch up to 2 heads ahead)

    with tile.TileContext(nc) as tc:
        with tc.tile_pool(name="const", bufs=1) as cpool:
            ident_bf = cpool.tile([128, 128], bf16, name="ident_bf")
            cf32 = cpool.tile([128, 62], f32, name="cf32")
            c32 = cpool.tile([MAX_N, 256], f32, name="c32")
            c1s1 = cpool.tile([128, 2 * TL * DH], bf16, name="c1s1")
            regf = cpool.tile([128, TL], f32, name="regf")
            qsb = cpool.tile([128, HPC * D], f32, name="qsb")
            bs_sb = cpool.tile([MAX_N, HPC], f32, name="bs_sb")
            bd_sb = cpool.tile([MAX_N, HPC], f32, name="bd_sb")
            kc_bufs = [
                cpool.tile([128, TL * D], f32, name=f"kc{i}") for i in range(NKV)
            ]
            vc_bufs = [
                cpool.tile([128, TL * D], f32, name=f"vc{i}") for i in range(NKV)
            ]
            # constant views
            inv128 = cf32[:, 0:16]
            nids = cf32[:, 16:48]
            ridx = cf32[:, 48:49]
            ones = cf32[:, 49:50]
            sgn2 = cf32[:, 50:52]
            sgnq = cf32[:, 52:54]
            rscal = cf32[:, 54:62]
            onehotP = c32[:, 0:128]
            atpre = c32[:, 128:256]
            c1t = c1s1[:, 0 : TL * DH]
            s1t = c1s1[:, TL * DH : 2 * TL * DH]

            # DMA issue order: small preamble-critical inputs/constants first,
            # then the head-0/1 K/V streams, then the bulk constants
            nc.sync.dma_start(
                regf[:], reg_d.ap().rearrange("(p tl) -> p tl", tl=TL)
            )
            nc.sync.dma_start(cf32[:], cf32_c.ap())
            nc.sync.dma_start(c32[:], c32_c.ap())
            nc.sync.dma_start(
                bs_sb[:],
                bs_d.ap().rearrange("(o h) -> o h", o=1).broadcast_to([MAX_N, HPC]),
            )
            nc.sync.dma_start(
                bd_sb[:],
                bd_d.ap().rearrange("(o h) -> o h", o=1).broadcast_to([MAX_N, HPC]),
            )
            nc.sync.dma_start(ident_bf[:], ident_bf_c.ap())
            nc.sync.dma_start(c1s1[:], c1s1_c.ap())
            nc.sync.dma_start(
                qsb.rearrange("p (h d) -> p h d", h=HPC),
                q_d.ap().rearrange("h p d -> p h d"),
            )
            for i in range(2):
                nc.sync.dma_start(
                    kc_bufs[i][:],
                    k_d.ap()[i].rearrange("(p x) d -> p (x d)", p=128),
                )
                nc.sync.dma_start(
                    vc_bufs[i][:],
                    v_d.ap()[i].rearrange("(p x) d -> p (x d)", p=128),
                )

            # ---- persistent per-core tables ----
            with tc.tile_pool(name="tables", bufs=1) as tpool:
                onehot = tpool.tile([128, TL * MAX_N], bf16, name="onehot")
                c2t = tpool.tile([128, TL * DH], bf16, name="c2t")
                s2st = tpool.tile([128, TL * DH], bf16, name="s2st")
                qa_all = tpool.tile([128, HPC * 128], bf16, name="qa_all")
                gpos = tpool.tile([128, 1], f32, name="gpos")
                delta32 = tpool.tile([MAX_N, HPC], f32, name="delta32")
                # persistent double buffers: the region one-hot block of Ka
                # (cols 96:128) and the ones column of vaug are head-invariant,
                # so each is written only into both buffers once
                ka_bufs = [
                    tpool.tile([128, TL * 128], bf16, name=f"ka{i}")
                    for i in range(NKV)
                ]
                vaug_bufs = [
                    tpool.tile([128, TL * (D + 1)], bf16, name=f"vaug{i}")
                    for i in range(NKV)
                ]

                with (
                    tc.tile_pool(name="pre_sb", bufs=1) as presb,
                    tc.tile_pool(name="work", bufs=3) as wpool,
                    tc.tile_pool(name="kat_sb", bufs=3) as katpool,
                    tc.tile_pool(name="attn", bufs=2) as apool,
                    tc.tile_pool(name="fin", bufs=2) as fpool,
                    tc.tile_pool(name="kt_ps", bufs=1, space="PSUM") as ktps,
                    tc.tile_pool(name="sc_ps", bufs=2, space="PSUM") as scps,
                    tc.tile_pool(name="av_ps", bufs=2, space="PSUM") as avps,
                ):
                    # one-hot region membership [p, tl, n]
                    oh_v = onehot.rearrange("p (tl n) -> p tl n", n=MAX_N)
                    nc.vector.tensor_tensor(
                        oh_v,
                        regf[:, :, None].broadcast_to([128, TL, MAX_N]),
                        nids[:, None, :].broadcast_to([128, TL, MAX_N]),
                        op=mybir.AluOpType.is_equal,
                    )
                    onesb = presb.tile([128, 1], bf16, name="onesb")
                    nc.vector.tensor_copy(onesb[:], ones[:])

                    # counts[n] = #t with regions == n+1 (order-invariant, so
                    # the contiguous layout works the same as tiled). PSUM is
                    # borrowed from the scores pool (cols 0:3 of one chunk).
                    pre_ps = scps.tile([128, 1024], f32, name="pre_ps", tag="scp")
                    cnt_ps = pre_ps[0:MAX_N, 0:1]
                    for t in range(TL):
                        nc.tensor.matmul(
                            cnt_ps,
                            oh_v[:, t, :],
                            onesb[:],
                            start=(t == 0),
                            stop=(t == TL - 1),
                        )
                    cnt_sb = presb.tile([MAX_N, 1], f32, name="cnt_sb")
                    nc.vector.tensor_copy(cnt_sb[:], cnt_ps)

                    # starts_exp[p] = sum_{n < p//4} counts[n]; gate by count>0
                    nc.tensor.matmul(
                        pre_ps[:, 1:2], atpre[:], cnt_sb[:], start=True, stop=True
                    )
                    nc.tensor.matmul(
                        pre_ps[:, 2:3], onehotP[:], cnt_sb[:], start=True, stop=True
                    )
                    gtm = presb.tile([128, 1], f32, name="gtm")
                    nc.vector.tensor_scalar(
                        gtm[:], pre_ps[:, 2:3], 0.0, None, op0=mybir.AluOpType.is_gt
                    )
                    nc.vector.tensor_mul(gpos[:], pre_ps[:, 1:2], gtm[:])

                    # ---- K-side region trig tables (h1 tables are inline
                    #      constants; h2 depends on regions). The whole chain
                    #      is split by tl-halves across DVE and Pool so the
                    #      serial reduce runs in half the time. ----
                    ang2 = presb.tile([128, TL * JH], f32, name="ang2")
                    a2_v = ang2.rearrange("p (tl j) -> p tl j", j=JH)
                    s2h = presb.tile([128, TL * JH], f32, name="s2h")
                    c2h = presb.tile([128, TL * JH], f32, name="c2h")
                    SPL = 20  # DVE takes 20 tl-columns, the slower Pool 12
                    for eng, t0, t1, nm in [
                        (nc.vector, 0, SPL, "t2a"),
                        (nc.gpsimd, SPL, TL, "t2b"),
                    ]:
                        ntl = t1 - t0
                        tls = slice(t0, t1)
                        cs = slice(t0 * JH, t1 * JH)
                        eng.tensor_tensor(
                            a2_v[:, tls, :],
                            regf[:, tls, None].broadcast_to([128, ntl, JH]),
                            inv128[:, None, :].broadcast_to([128, ntl, JH]),
                            op=mybir.AluOpType.mult,
                        )
                        # the reduce runs on DVE for both halves: the scalar-
                        # flavored ops are not legal Pool opcodes on HW
                        _emit_sincos(
                            nc, mybir, presb, AF, ang2[:, cs], s2h[:, cs],
                            c2h[:, cs], ntl * JH, nm, eng=nc.vector,
                        )
                        # pair-expand to full 32 dims (d=2j,2j+1 share coeffs)
                        eng.tensor_copy(
                            c2t.rearrange("p (tl j e) -> p tl j e", tl=TL, e=2)[
                                :, tls
                            ],
                            c2h.rearrange("p (tl j) -> p tl j", j=JH)[
                                :, tls, :, None
                            ].broadcast_to([128, ntl, JH, 2]),
                        )
                        # signed sin: s2st[.., 2j] = -s2, [.., 2j+1] = +s2
                        eng.tensor_tensor(
                            s2st.rearrange("p (tl j e) -> p tl j e", tl=TL, e=2)[
                                :, tls
                            ],
                            s2h.rearrange("p (tl j) -> p tl j", j=JH)[
                                :, tls, :, None
                            ].broadcast_to([128, ntl, JH, 2]),
                            sgn2[:, None, None, :].broadcast_to([128, ntl, JH, 2]),
                            op=mybir.AluOpType.mult,
                        )

                    # ================= per-head emitters =================
                    cs1_v = c1s1.rearrange(
                        "p (tl s d) -> p tl s d", tl=TL, s=2
                    )  # [p, tl, {cos,sin}, 32]
                    c2_v = c2t.rearrange("p (tl d) -> p tl d", tl=TL)
                    s2_v = s2st.rearrange("p (tl d) -> p tl d", tl=TL)
                    tiles = {}

                    def emit_kpath_pre(h):
                        # prefetch K for head h+2 (heads 0-1 loaded at top);
                        # the V prefetch lives in emit_kpath_tr, which runs one
                        # stage later — emitting it here would queue the
                        # overwrite of buffer (h+2)%NKV ahead of tr(h+1)'s read
                        # of that same buffer
                        if h + 2 < HPC:
                            nc.sync.dma_start(
                                kc_bufs[(h + 2) % NKV][:],
                                k_d.ap()[h + 2].rearrange(
                                    "(p x) d -> p (x d)", p=128
                                ),
                            )
                        kc = kc_bufs[h % NKV]
                        vc = vc_bufs[h % NKV]

                        # cast K to fp16 (ACT for the first two heads, while
                        # Pool is loaded later in steady state)
                        kbf = wpool.tile([128, TL * D], bf16, name="kbf", tag="kbf")
                        if h < 2:
                            nc.scalar.copy(kbf[:], kc[:])
                        else:
                            nc.gpsimd.tensor_copy(kbf[:], kc[:])
                        kb_v = kbf.rearrange("p (tl d) -> p tl d", tl=TL)

                        # assemble the 128-dim contraction tiles Ka
                        ka = ka_bufs[h % NKV]
                        ka_v = ka.rearrange("p (tl c) -> p tl c", tl=TL)
                        nc.vector.tensor_mul(
                            ka_v[:, :, 0:DH], kb_v[:, :, 0:DH], cs1_v[:, :, 0, :]
                        )
                        nc.vector.tensor_mul(
                            ka_v[:, :, DH : 2 * DH],
                            kb_v[:, :, 0:DH],
                            cs1_v[:, :, 1, :],
                        )
                        nc.vector.tensor_mul(
                            ka_v[:, :, 2 * DH : 96], kb_v[:, :, DH:D], c2_v
                        )
                        ktmp = wpool.tile(
                            [128, TL * DH], bf16, name="ktmp", tag="ktmp"
                        )
                        kswap = kb_v[:, :, DH:D].rearrange(
                            "p tl (j e) -> p tl j e", e=2
                        )[:, :, :, ::-1]
                        nc.vector.tensor_tensor(
                            ktmp.rearrange("p (tl j e) -> p tl j e", tl=TL, e=2),
                            kswap,
                            s2_v.rearrange("p tl (j e) -> p tl j e", e=2),
                            op=mybir.AluOpType.mult,
                        )
                        nc.vector.tensor_add(
                            ka_v[:, :, 2 * DH : 96],
                            ka_v[:, :, 2 * DH : 96],
                            ktmp.rearrange("p (tl d) -> p tl d", tl=TL),
                        )
                        # region one-hot block: head-invariant, written only
                        # into each persistent Ka buffer once
                        if h < NKV:
                            nc.vector.tensor_copy(
                                ka_v[:, :, 96:128],
                                onehot.rearrange("p (tl n) -> p tl n", n=MAX_N),
                            )
                        tiles["vc", h] = vc

                    def emit_kpath_tr(h):
                        if h + 2 < HPC:
                            nc.sync.dma_start(
                                vc_bufs[(h + 2) % NKV][:],
                                v_d.ap()[h + 2].rearrange(
                                    "(p x) d -> p (x d)", p=128
                                ),
                            )
                        ka_v = ka_bufs[h % NKV].rearrange("p (tl c) -> p tl c", tl=TL)
                        vc = tiles["vc", h]
                        # transpose Ka tiles -> KaT [c=128, t] (stride-32 sets)
                        kat = katpool.tile([128, T], bf16, name="kat", tag="kat")
                        for g in range(2):
                            ktp = ktps.tile(
                                [128, 2048], bf16, name="ktp", tag="ktp"
                            )
                            for i in range(16):
                                l = g * 16 + i
                                nc.tensor.transpose(
                                    ktp[:, i * 128 : (i + 1) * 128],
                                    ka_v[:, l, :],
                                    ident_bf[:],
                                )
                            nc.vector.tensor_copy(
                                kat[:, g * 2048 : (g + 1) * 2048], ktp[:]
                            )
                        tiles["kat", h] = kat

                        # V: cast + ones column (persistent per buffer)
                        vaug = vaug_bufs[h % NKV]
                        va_v = vaug.rearrange("p (tl d) -> p tl d", tl=TL)
                        vc_v = vc.rearrange("p (tl d) -> p tl d", tl=TL)
                        if h < 2:
                            nc.gpsimd.tensor_copy(va_v[:, :, 0:D], vc_v)
                        else:
                            nc.gpsimd.tensor_copy(
                                va_v[:, 0:22, 0:D], vc_v[:, 0:22, :]
                            )
                            nc.scalar.copy(va_v[:, 22:TL, 0:D], vc_v[:, 22:TL, :])
                        if h < NKV:
                            nc.gpsimd.memset(va_v[:, :, D : D + 1], 1.0)
                        tiles["vaug", h] = vaug

                    def emit_qk(h):
                        kat = tiles["kat", h]
                        at = apool.tile([128, T], bf16, name="at", tag="at")
                        for g in range(4):
                            scp = scps.tile(
                                [128, 1024], f32, name="scp", tag="scp"
                            )
                            for i in range(8):
                                l = g * 8 + i
                                nc.tensor.matmul(
                                    scp[:, i * 128 : (i + 1) * 128],
                                    kat[:, l * 128 : (l + 1) * 128],
                                    qa_all[:, h * 128 : (h + 1) * 128],
                                    start=True,
                                    stop=True,
                                )
                            nc.scalar.activation(
                                at[:, g * 1024 : (g + 1) * 1024], scp[:], AF.Exp
                            )
                        tiles["at", h] = at

                    def emit_av(h):
                        at = tiles["at", h]
                        va_v = tiles["vaug", h].rearrange(
                            "p (tl d) -> p tl d", tl=TL
                        )
                        avp = avps.tile([128, 128], f32, name="avp", tag="avp")
                        for l in range(TL):
                            nc.tensor.matmul(
                                avp[:, 0 : D + 1],
                                at[:, l * 128 : (l + 1) * 128],
                                va_v[:, l, :],
                                start=(l == 0),
                                stop=(l == TL - 1),
                            )
                        tiles["avp", h] = avp

                    def emit_finish(h):
                        # normalize and store (already [p, d] layout); GPSIMD
                        # cannot touch PSUM on real HW, so reciprocal runs on
                        # DVE and the scaled copy on ACT
                        avp = tiles["avp", h]
                        rden = fpool.tile([128, 1], f32, name="rden", tag="rden")
                        nc.vector.reciprocal(rden[:], avp[:, D : D + 1])
                        osb = fpool.tile([128, D], f32, name="osb", tag="osb")
                        nc.scalar.activation(
                            osb[:], avp[:, 0:D], AF.Copy, scale=rden[:]
                        )
                        nc.sync.dma_start(out_d.ap()[h], osb[:])

                    # ---- Q-side tables (scaled by 1/8 = 1/sqrt(D)) ----
                    angq = presb.tile([128, DH], f32, name="angq")
                    nc.vector.tensor_scalar_mul(angq[:, 0:JH], inv128[:], gpos[:])
                    nc.vector.tensor_scalar_mul(angq[:, JH:DH], inv128[:], ridx[:])
                    sinq = presb.tile([128, DH], f32, name="sinq")
                    cosq = presb.tile([128, DH], f32, name="cosq")
                    _emit_sincos(
                        nc, mybir, presb, AF, angq[:], sinq[:], cosq[:], DH, "tq"
                    )
                    nc.scalar.mul(cosq[:], cosq[:], 0.125)
                    nc.scalar.mul(sinq[:], sinq[:], 0.125)

                    # ---- rotate Q (split across DVE and the idle Pool by
                    #      head-halves; Pool only runs TT ops, which are legal)
                    qs_v = qsb.rearrange("p (h j e) -> p h j e", h=HPC, e=2)
                    qrot = presb.tile([128, HPC * D], f32, name="qrot")
                    qr_v = qrot.rearrange("p (h j e) -> p h j e", h=HPC, e=2)
                    qtm = presb.tile([128, HPC * DH], f32, name="qtm")
                    qtm_v = qtm.rearrange("p (h j) -> p h j", h=HPC)
                    # cosq/sinq are [128, 32]: one value per rotation pair,
                    # 16 pairs per half concatenated
                    HH = HPC // 2
                    for eng, hs in [
                        (nc.vector, slice(0, HH)),
                        (nc.gpsimd, slice(HH, HPC)),
                    ]:
                        sq_b = sinq[:, None, :].broadcast_to([128, HH, DH])
                        cq_b = cosq[:, None, :, None].broadcast_to(
                            [128, HH, DH, 2]
                        )
                        eng.tensor_tensor(
                            qr_v[:, hs], qs_v[:, hs], cq_b,
                            op=mybir.AluOpType.mult,
                        )
                        eng.tensor_tensor(
                            qtm_v[:, hs], qs_v[:, hs, :, 1], sq_b,
                            op=mybir.AluOpType.mult,
                        )
                        eng.tensor_tensor(
                            qr_v[:, hs, :, 0], qr_v[:, hs, :, 0], qtm_v[:, hs],
                            op=mybir.AluOpType.subtract,
                        )
                        eng.tensor_tensor(
                            qtm_v[:, hs], qs_v[:, hs, :, 0], sq_b,
                            op=mybir.AluOpType.mult,
                        )
                        eng.tensor_tensor(
                            qr_v[:, hs, :, 1], qr_v[:, hs, :, 1], qtm_v[:, hs],
                            op=mybir.AluOpType.add,
                        )

                    # ---- per-head bias scale delta_h = bias_same - bias_diff
                    nc.vector.tensor_sub(delta32[:], bs_sb[:], bd_sb[:])

                    # ---- assemble Qa per head: rows [q'_h1 | swapneg(q'_h1) |
                    #      q'_h2 | delta_h*onehotP] ----
                    qpre = presb.tile([128, HPC * 96], bf16, name="qpre")
                    qp_v = qpre.rearrange("p (h c) -> p h c", h=HPC)
                    qr_h = qrot.rearrange("p (h d) -> p h d", h=HPC)
                    nc.vector.tensor_copy(qp_v[:, :, 0:DH], qr_h[:, :, 0:DH])
                    nc.vector.tensor_copy(qp_v[:, :, 2 * DH : 96], qr_h[:, :, DH:D])
                    # swapneg: col 32+2j = +q'[2j+1], col 32+2j+1 = -q'[2j]
                    qswap = qr_h[:, :, 0:DH].rearrange(
                        "p h (j e) -> p h j e", e=2
                    )[:, :, :, ::-1]
                    nc.vector.tensor_tensor(
                        qp_v[:, :, DH : 2 * DH].rearrange(
                            "p h (j e) -> p h j e", e=2
                        ),
                        qswap,
                        sgnq[:, None, None, :].broadcast_to([128, HPC, JH, 2]),
                        op=mybir.AluOpType.mult,
                    )
                    # transpose all 8 rot-blocks through one borrowed psum tile
                    qtp = ktps.tile([128, 2048], bf16, name="qtp", tag="ktp")
                    for h in range(HPC):
                        nc.tensor.transpose(
                            qtp[0:96, h * 128 : (h + 1) * 128],
                            qp_v[:, h, :],
                            ident_bf[:],
                        )
                        nc.scalar.activation(
                            qa_all[96:128, h * 128 : (h + 1) * 128],
                            onehotP[:],
                            AF.Copy,
                            scale=delta32[:, h : h + 1],
                        )
                    nc.vector.tensor_copy(qa_all[0:96, :], qtp[0:96, 0:1024])

                    # ============== main pipelined loop ==============
                    emit_kpath_pre(0)
                    emit_kpath_pre(1)
                    emit_kpath_tr(0)
                    for h in range(HPC):
                        emit_qk(h)
                        if h + 2 < HPC:
                            emit_kpath_pre(h + 2)
                        if h + 1 < HPC:
                            emit_kpath_tr(h + 1)
                        emit_av(h)
                        if h >= 1:
                            emit_finish(h - 1)
                    emit_finish(HPC - 1)

    if split_waits:
        _split_waits(nc)
    return nc


def _get_program():
    if "nc" not in _cache:
        _cache["nc"] = _build_program()
    return _cache["nc"]


TRACE = False  # test.py sets True to capture NTFF profile + exec_time_ns
LAST_RESULT = None


def kernel(
    query_q,
    x_k,
    x_v,
    regions,
    t_mask=None,
    n_mask=None,
    max_n=None,
    bias_same=None,
    bias_diff=None,
    **_unused,
):
    from concourse import bass_utils

    nc = _get_program()

    query_q = np.asarray(query_q, dtype=np.float32)
    x_k = np.asarray(x_k, dtype=np.float32)
    x_v = np.asarray(x_v, dtype=np.float32)
    regions_f = np.asarray(regions).astype(np.float32)
    bias_same = np.asarray(bias_same, dtype=np.float32)
    bias_diff = np.asarray(bias_diff, dtype=np.float32)

    in_maps = []
    for core in range(NCORES):
        b = core // 2
        h0 = (core % 2) * HPC
        in_maps.append(
            {
                "q": np.ascontiguousarray(query_q[b, h0 : h0 + HPC]),
                "k": np.ascontiguousarray(x_k[b, h0 : h0 + HPC]),
                "v": np.ascontiguousarray(x_v[b, h0 : h0 + HPC]),
                "regions_f": np.ascontiguousarray(regions_f[b]),
                "bias_same8": np.ascontiguousarray(bias_same[h0 : h0 + HPC]),
                "bias_diff8": np.ascontiguousarray(bias_diff[h0 : h0 + HPC]),
            }
        )

    global LAST_RESULT
    res = bass_utils.run_bass_kernel_spmd(
        nc, in_maps, core_ids=list(range(NCORES)), trace=TRACE
    )
    LAST_RESULT = res

    out = np.empty((B, H, P, D), np.float32)
    for core in range(NCORES):
        b = core // 2
        h0 = (core % 2) * HPC
        out[b, h0 : h0 + HPC] = res.results[core]["out"]
    return out
